# revision 18
# baseline (speedup 1.0000x reference)
"""Trainium2 Bass kernel for CascadedPathEncoder.

Reference computation (per sample b):
    h_0 = relu(W_0 @ [0_256; wp_0] + b_0)
    h_p = relu(W_p @ [h_{p-1}; wp_p] + b_p)      p = 1..31
    out[b] = concat_p h_p                         -> [8192, 8192]

Strategy: pure data parallel over 8 NeuronCores (1024 batch rows each).
Per core the hidden state is kept transposed in SBUF as two bf16
[128, 1024] chunks (partition dim = hidden index). Each step runs, per
512-column batch tile and per output chunk m, three matmuls that
accumulate in PSUM: K=4 (the wp contribution, lhsT from W[:, :, 256:260])
and two K=128 chunks (lhsT from W[:, :, :256]). Bias + relu are fused on
the Activation engine (m=0) and Vector engine (m=1), writing the new
bf16 state chunk, which is also DMA'd to DRAM as the step's output
slice. Host un-transposes / re-assembles the full [8192, 8192] f32.
"""

import numpy as np
import ml_dtypes

BF16 = ml_dtypes.bfloat16
P = 32          # scan steps
PD = 4          # point dim
H = 256         # hidden dim
B = 8192        # global batch
NCORES = 8
BS = B // NCORES  # 1024 rows per core
TN = 512        # matmul moving free dim (one PSUM bank of f32; ISA max)
NT = BS // TN   # batch tiles per core

_CACHE = {}


def _build_nc():
    from contextlib import ExitStack

    import concourse.bass as bass
    import concourse.tile as tile
    from concourse import bacc, mybir

    dt = mybir.dt
    ts = bass.ts

    nc = bacc.Bacc(
        "TRN2", target_bir_lowering=False, debug=False, num_devices=NCORES
    )
    WH_CHUNK = 8  # steps per wh DMA chunk (pipeline weight loads)
    PD_CHUNK = 4  # steps per wx/pdx DMA chunk
    wh = nc.dram_tensor("wh", [128, P, 2, 256], dt.bfloat16, kind="ExternalInput").ap()
    wx = nc.dram_tensor("wx", [PD, P, 2, 128], dt.bfloat16, kind="ExternalInput").ap()
    pdx = nc.dram_tensor("pdx", [PD, P, BS], dt.bfloat16, kind="ExternalInput").ap()
    bias = nc.dram_tensor("bias", [128, P, 2], dt.float32, kind="ExternalInput").ap()
    out = nc.dram_tensor("out", [P, 2, 128, BS], dt.bfloat16, kind="ExternalOutput").ap()

    with tile.TileContext(nc) as tc, ExitStack() as ctx:
        const = ctx.enter_context(tc.tile_pool(name="const", bufs=1))
        state = ctx.enter_context(tc.tile_pool(name="state", bufs=2))
        psum = ctx.enter_context(tc.tile_pool(name="psum", bufs=2, space="PSUM"))

        # Step 0 needs wx/pdx chunk 0, step 1 additionally wh chunk 0; order
        # the DMAs by first-use. wx/pdx live on 4 partitions only (matmul
        # base partition must be 0/32/64), so their DMAs run far below peak
        # — chunk them by step range so early steps aren't gated on the
        # full transfer.
        wx_sb = const.tile([PD, P, 2, 128], dt.bfloat16)
        pdx_sb = const.tile([PD, P, BS], dt.bfloat16)
        b_sb = const.tile([128, P, 2], dt.float32)
        wh_sb = const.tile([128, P, 2, 256], dt.bfloat16)

        nc.sync.dma_start(out=wx_sb[:, 0:PD_CHUNK, :, :], in_=wx[:, 0:PD_CHUNK, :, :])
        nc.sync.dma_start(out=pdx_sb[:, 0:PD_CHUNK, :], in_=pdx[:, 0:PD_CHUNK, :])
        nc.sync.dma_start(out=b_sb[:], in_=bias[:])
        nc.sync.dma_start(
            out=wh_sb[:, 0:WH_CHUNK, :, :], in_=wh[:, 0:WH_CHUNK, :, :]
        )
        for g in range(1, P // PD_CHUNK):
            sl = slice(g * PD_CHUNK, (g + 1) * PD_CHUNK)
            nc.sync.dma_start(out=wx_sb[:, sl, :, :], in_=wx[:, sl, :, :])
            nc.sync.dma_start(out=pdx_sb[:, sl, :], in_=pdx[:, sl, :])
        for g in range(1, P // WH_CHUNK):
            sl = slice(g * WH_CHUNK, (g + 1) * WH_CHUNK)
            nc.sync.dma_start(out=wh_sb[:, sl, :, :], in_=wh[:, sl, :, :])

        h_prev = [[None, None] for _ in range(NT)]
        for p in range(P):
            ps = [
                [
                    psum.tile(
                        [128, TN],
                        dt.float32,
                        tag=f"ps_t{t}m{m}",
                        name=f"ps_p{p}t{t}m{m}",
                    )
                    for m in range(2)
                ]
                for t in range(NT)
            ]
            # K=4 wp contribution opens each accumulation group. t is
            # innermost everywhere so consecutive matmuls share lhsT (one
            # weight load covers NT matmuls). k outer / m inner for the
            # K=128 chunks: the k=0 matmuls only need the previous step's
            # m=0 relu, giving the m=1 relu a longer window to complete.
            for m in range(2):
                for t in range(NT):
                    nc.tensor.matmul(
                        ps[t][m][:],
                        lhsT=wx_sb[:, p, m, :],
                        rhs=pdx_sb[:, p, ts(t, TN)],
                        start=True,
                        stop=(p == 0),
                    )
            if p > 0:
                for k in range(2):
                    for m in range(2):
                        for t in range(NT):
                            nc.tensor.matmul(
                                ps[t][m][:],
                                lhsT=wh_sb[:, p, k, ts(m, 128)],
                                rhs=h_prev[t][k][:],
                                start=False,
                                stop=(k == 1),
                            )
            for t in range(NT):
                h_new = []
                for m in range(2):
                    hn = state.tile(
                        [128, TN],
                        dt.bfloat16,
                        tag=f"h_t{t}m{m}",
                        name=f"h_p{p}t{t}m{m}",
                    )
                    if m == 0:
                        nc.scalar.activation(
                            hn[:],
                            ps[t][m][:],
                            mybir.ActivationFunctionType.Relu,
                            bias=b_sb[:, p, m : m + 1],
                            scale=1.0,
                        )
                    else:
                        nc.vector.tensor_scalar(
                            hn[:],
                            ps[t][m][:],
                            scalar1=b_sb[:, p, m : m + 1],
                            scalar2=0.0,
                            op0=mybir.AluOpType.add,
                            op1=mybir.AluOpType.max,
                        )
                    nc.sync.dma_start(out=out[p, m, :, ts(t, TN)], in_=hn[:])
                    h_new.append(hn)
                h_prev[t] = h_new

    nc.compile()
    return nc


def _get_nc():
    if "nc" not in _CACHE:
        _CACHE["nc"] = _build_nc()
    return _CACHE["nc"]


def _pack_inputs(path_data, W, b):
    """Host-side packing into the DRAM layouts the kernel expects."""
    # lhsT for the two K=128 chunks: wh[kk, p, k, jj] = W[p, jj, 128k+kk]
    wh_np = np.ascontiguousarray(
        W[:, :, :H].reshape(P, H, 2, 128).transpose(3, 0, 2, 1)
    ).astype(BF16)
    # lhsT for the K=4 chunk: wx[r, p, m, j] = W[p, 128m+j, 256+r]
    wx_np = np.ascontiguousarray(
        W[:, :, H:].reshape(P, 2, 128, PD).transpose(3, 0, 1, 2)
    ).astype(BF16)
    # bias[j, p, m] = b[p, 128m+j]
    b_np = np.ascontiguousarray(b.reshape(P, 2, 128).transpose(2, 0, 1)).astype(
        np.float32
    )
    # per-core rhs for the K=4 chunk: pdx[r, p, bb] = path_data[c*BS+bb, 4p+r]
    pdx_all = [
        np.ascontiguousarray(
            path_data[c * BS : (c + 1) * BS].reshape(BS, P, PD).transpose(2, 1, 0)
        ).astype(BF16)
        for c in range(NCORES)
    ]
    return wh_np, wx_np, b_np, pdx_all


def kernel(path_data, W, b):
    from concourse.bass_utils import run_bass_kernel_spmd

    path_data = np.asarray(path_data, dtype=np.float32)
    W = np.asarray(W, dtype=np.float32)
    b = np.asarray(b, dtype=np.float32)

    wh_np, wx_np, b_np, pdx_all = _pack_inputs(path_data, W, b)
    in_maps = [
        {"wh": wh_np, "wx": wx_np, "bias": b_np, "pdx": pdx_all[c]}
        for c in range(NCORES)
    ]

    nc = _get_nc()
    res = run_bass_kernel_spmd(nc, in_maps, core_ids=list(range(NCORES)))

    full = np.concatenate(
        [
            np.asarray(r["out"])
            .transpose(3, 0, 1, 2)
            .reshape(BS, P * H)
            .astype(np.float32)
            for r in res.results
        ],
        axis=0,
    )
    return full


# revision 24
# speedup vs baseline: 1.0415x; 1.0415x over previous
"""Trainium2 Bass kernel for CascadedPathEncoder.

Reference computation (per sample b):
    h_0 = relu(W_0 @ [0_256; wp_0] + b_0)
    h_p = relu(W_p @ [h_{p-1}; wp_p] + b_p)      p = 1..31
    out[b] = concat_p h_p                         -> [8192, 8192]

Strategy: pure data parallel over 8 NeuronCores (1024 batch rows each).
Per core the hidden state is kept transposed in SBUF as two bf16
[128, 1024] chunks (partition dim = hidden index). Each step runs, per
512-column batch tile and per output chunk m, three matmuls that
accumulate in PSUM: K=4 (the wp contribution, lhsT from W[:, :, 256:260])
and two K=128 chunks (lhsT from W[:, :, :256]). Bias + relu are fused on
the Activation engine (m=0) and Vector engine (m=1), writing the new
bf16 state chunk, which is also DMA'd to DRAM as the step's output
slice. Host un-transposes / re-assembles the full [8192, 8192] f32.
"""

import numpy as np
import ml_dtypes

BF16 = ml_dtypes.bfloat16
P = 32          # scan steps
PD = 4          # point dim
H = 256         # hidden dim
B = 8192        # global batch
NCORES = 8
BS = B // NCORES  # 1024 rows per core
TN = 512        # matmul moving free dim (one PSUM bank of f32; ISA max)
NT = BS // TN   # batch tiles per core

_CACHE = {}


def _build_nc():
    from contextlib import ExitStack

    import concourse.bass as bass
    import concourse.tile as tile
    from concourse import bacc, mybir

    dt = mybir.dt
    ts = bass.ts

    nc = bacc.Bacc(
        "TRN2", target_bir_lowering=False, debug=False, num_devices=NCORES
    )
    WH_CHUNK = 4  # steps per wh DMA chunk (pipeline weight loads)
    WX_CHUNK = 8  # steps per wx DMA chunk
    wh = nc.dram_tensor("wh", [128, P, 2, 256], dt.bfloat16, kind="ExternalInput").ap()
    # wx is stored as zero-padded K=64 lhsT blocks so both wx and pdx can be
    # read at matmul-legal base partitions (0/64) while their DMAs stay full
    # 128-partition width: wx[4p+r, p, m, j] = W[p, 128m+j, 256+r], zero
    # elsewhere; pdx[4p+r, b] = path_data[b, 4p+r].
    wx = nc.dram_tensor("wx", [128, P, 2, 128], dt.bfloat16, kind="ExternalInput").ap()
    pdx = nc.dram_tensor("pdx", [128, BS], dt.bfloat16, kind="ExternalInput").ap()
    bias = nc.dram_tensor("bias", [128, P, 2], dt.float32, kind="ExternalInput").ap()
    out = nc.dram_tensor("out", [P, 2, 128, BS], dt.bfloat16, kind="ExternalOutput").ap()

    with tile.TileContext(nc) as tc, ExitStack() as ctx:
        const = ctx.enter_context(tc.tile_pool(name="const", bufs=1))
        state = ctx.enter_context(tc.tile_pool(name="state", bufs=2))
        psum = ctx.enter_context(tc.tile_pool(name="psum", bufs=2, space="PSUM"))

        # Order the input DMAs by first-use; all transfers are full width.
        wx_sb = const.tile([128, P, 2, 128], dt.bfloat16)
        pdx_sb = const.tile([128, BS], dt.bfloat16)
        b_sb = const.tile([128, P, 2], dt.float32)
        wh_sb = const.tile([128, P, 2, 256], dt.bfloat16)

        nc.sync.dma_start(out=pdx_sb[:], in_=pdx[:])
        nc.sync.dma_start(
            out=wx_sb[:, 0:WX_CHUNK, :, :], in_=wx[:, 0:WX_CHUNK, :, :]
        )
        nc.sync.dma_start(out=b_sb[:], in_=bias[:])
        nc.sync.dma_start(
            out=wh_sb[:, 0:WH_CHUNK, :, :], in_=wh[:, 0:WH_CHUNK, :, :]
        )
        for g in range(1, P // WH_CHUNK):
            sl = slice(g * WH_CHUNK, (g + 1) * WH_CHUNK)
            nc.sync.dma_start(out=wh_sb[:, sl, :, :], in_=wh[:, sl, :, :])
            if g < P // WX_CHUNK:
                slx = slice(g * WX_CHUNK, (g + 1) * WX_CHUNK)
                nc.sync.dma_start(out=wx_sb[:, slx, :, :], in_=wx[:, slx, :, :])

        h_prev = [[None, None] for _ in range(NT)]
        for p in range(P):
            ps = [
                [
                    psum.tile(
                        [128, TN],
                        dt.float32,
                        tag=f"ps_t{t}m{m}",
                        name=f"ps_p{p}t{t}m{m}",
                    )
                    for m in range(2)
                ]
                for t in range(NT)
            ]
            # K=4 wp contribution opens each accumulation group. t is
            # innermost everywhere so consecutive matmuls share lhsT (one
            # weight load covers NT matmuls). k outer / m inner for the
            # K=128 chunks: the k=0 matmuls only need the previous step's
            # m=0 relu, giving the m=1 relu a longer window to complete.
            base = 64 * (p // 16)
            for m in range(2):
                for t in range(NT):
                    nc.tensor.matmul(
                        ps[t][m][:],
                        lhsT=wx_sb[base : base + 64, p, m, :],
                        rhs=pdx_sb[base : base + 64, ts(t, TN)],
                        start=True,
                        stop=(p == 0),
                    )
            if p > 0:
                for k in range(2):
                    for m in range(2):
                        for t in range(NT):
                            nc.tensor.matmul(
                                ps[t][m][:],
                                lhsT=wh_sb[:, p, k, ts(m, 128)],
                                rhs=h_prev[t][k][:],
                                start=False,
                                stop=(k == 1),
                            )
            for t in range(NT):
                h_new = []
                for m in range(2):
                    hn = state.tile(
                        [128, TN],
                        dt.bfloat16,
                        tag=f"h_t{t}m{m}",
                        name=f"h_p{p}t{t}m{m}",
                    )
                    if m == 0:
                        nc.scalar.activation(
                            hn[:],
                            ps[t][m][:],
                            mybir.ActivationFunctionType.Relu,
                            bias=b_sb[:, p, m : m + 1],
                            scale=1.0,
                        )
                    else:
                        nc.vector.tensor_scalar(
                            hn[:],
                            ps[t][m][:],
                            scalar1=b_sb[:, p, m : m + 1],
                            scalar2=0.0,
                            op0=mybir.AluOpType.add,
                            op1=mybir.AluOpType.max,
                        )
                    nc.sync.dma_start(out=out[p, m, :, ts(t, TN)], in_=hn[:])
                    h_new.append(hn)
                h_prev[t] = h_new

    nc.compile()
    return nc


def _get_nc():
    if "nc" not in _CACHE:
        _CACHE["nc"] = _build_nc()
    return _CACHE["nc"]


def _pack_inputs(path_data, W, b):
    """Host-side packing into the DRAM layouts the kernel expects."""
    # lhsT for the two K=128 chunks: wh[kk, p, k, jj] = W[p, jj, 128k+kk]
    wh_np = np.ascontiguousarray(
        W[:, :, :H].reshape(P, H, 2, 128).transpose(3, 0, 2, 1)
    ).astype(BF16)
    # zero-padded K=64 lhsT blocks for the wp chunk:
    # wx[4p+r, p, m, j] = W[p, 128m+j, 256+r]
    wx_np = np.zeros((128, P, 2, 128), dtype=BF16)
    wxs = W[:, :, H:].reshape(P, 2, 128, PD).transpose(3, 0, 1, 2).astype(BF16)
    for p in range(P):
        wx_np[4 * p : 4 * p + 4, p] = wxs[:, p]
    # bias[j, p, m] = b[p, 128m+j]
    b_np = np.ascontiguousarray(b.reshape(P, 2, 128).transpose(2, 0, 1)).astype(
        np.float32
    )
    # per-core rhs for the wp chunk: pdx[4p+r, bb] = path_data[c*BS+bb, 4p+r]
    pdx_all = [
        np.ascontiguousarray(path_data[c * BS : (c + 1) * BS].T).astype(BF16)
        for c in range(NCORES)
    ]
    return wh_np, wx_np, b_np, pdx_all


def kernel(path_data, W, b):
    from concourse.bass_utils import run_bass_kernel_spmd

    path_data = np.asarray(path_data, dtype=np.float32)
    W = np.asarray(W, dtype=np.float32)
    b = np.asarray(b, dtype=np.float32)

    wh_np, wx_np, b_np, pdx_all = _pack_inputs(path_data, W, b)
    in_maps = [
        {"wh": wh_np, "wx": wx_np, "bias": b_np, "pdx": pdx_all[c]}
        for c in range(NCORES)
    ]

    nc = _get_nc()
    res = run_bass_kernel_spmd(nc, in_maps, core_ids=list(range(NCORES)))

    full = np.concatenate(
        [
            np.asarray(r["out"])
            .transpose(3, 0, 1, 2)
            .reshape(BS, P * H)
            .astype(np.float32)
            for r in res.results
        ],
        axis=0,
    )
    return full


# revision 25
# speedup vs baseline: 1.0807x; 1.0377x over previous
"""Trainium2 Bass kernel for CascadedPathEncoder.

Reference computation (per sample b):
    h_0 = relu(W_0 @ [0_256; wp_0] + b_0)
    h_p = relu(W_p @ [h_{p-1}; wp_p] + b_p)      p = 1..31
    out[b] = concat_p h_p                         -> [8192, 8192]

Strategy: pure data parallel over 8 NeuronCores (1024 batch rows each).
Per core the hidden state is kept transposed in SBUF as two bf16
[128, 1024] chunks (partition dim = hidden index). Each step runs, per
512-column batch tile and per output chunk m, three matmuls that
accumulate in PSUM: K=4 (the wp contribution, lhsT from W[:, :, 256:260])
and two K=128 chunks (lhsT from W[:, :, :256]). Bias + relu are fused on
the Activation engine (m=0) and Vector engine (m=1), writing the new
bf16 state chunk, which is also DMA'd to DRAM as the step's output
slice. Host un-transposes / re-assembles the full [8192, 8192] f32.
"""

import numpy as np
import ml_dtypes

BF16 = ml_dtypes.bfloat16
P = 32          # scan steps
PD = 4          # point dim
H = 256         # hidden dim
B = 8192        # global batch
NCORES = 8
BS = B // NCORES  # 1024 rows per core
TN = 512        # matmul moving free dim (one PSUM bank of f32; ISA max)
NT = BS // TN   # batch tiles per core

_CACHE = {}


def _build_nc():
    from contextlib import ExitStack

    import concourse.bass as bass
    import concourse.tile as tile
    from concourse import bacc, mybir

    dt = mybir.dt
    ts = bass.ts

    nc = bacc.Bacc(
        "TRN2", target_bir_lowering=False, debug=False, num_devices=NCORES
    )
    WH_CHUNK = 4  # steps per wh DMA chunk (pipeline weight loads)
    WX_CHUNK = 8  # steps per wx DMA chunk
    wh = nc.dram_tensor("wh", [128, P, 2, 256], dt.bfloat16, kind="ExternalInput").ap()
    # wx is stored as zero-padded K=64 lhsT blocks so both wx and pdx can be
    # read at matmul-legal base partitions (0/64) while their DMAs stay full
    # 128-partition width: wx[4p+r, p, m, j] = W[p, 128m+j, 256+r], zero
    # elsewhere; pdx[4p+r, b] = path_data[b, 4p+r].
    wx = nc.dram_tensor("wx", [128, P, 2, 128], dt.bfloat16, kind="ExternalInput").ap()
    pdx = nc.dram_tensor("pdx", [128, BS], dt.bfloat16, kind="ExternalInput").ap()
    bias = nc.dram_tensor("bias", [128, P, 2], dt.float32, kind="ExternalInput").ap()
    out = nc.dram_tensor("out", [P, 2, 128, BS], dt.bfloat16, kind="ExternalOutput").ap()

    with tile.TileContext(nc) as tc, ExitStack() as ctx:
        const = ctx.enter_context(tc.tile_pool(name="const", bufs=1))
        state = ctx.enter_context(tc.tile_pool(name="state", bufs=2))
        psum = ctx.enter_context(tc.tile_pool(name="psum", bufs=2, space="PSUM"))

        # Order the input DMAs by first-use; all transfers are full width.
        wx_sb = const.tile([128, P, 2, 128], dt.bfloat16)
        pdx_sb = const.tile([128, BS], dt.bfloat16)
        b_sb = const.tile([128, P, 2], dt.float32)
        wh_sb = const.tile([128, P, 2, 256], dt.bfloat16)

        nc.sync.dma_start(out=pdx_sb[:], in_=pdx[:])
        nc.sync.dma_start(
            out=wx_sb[:, 0:WX_CHUNK, :, :], in_=wx[:, 0:WX_CHUNK, :, :]
        )
        nc.sync.dma_start(out=b_sb[:], in_=bias[:])
        nc.sync.dma_start(
            out=wh_sb[:, 0:WH_CHUNK, :, :], in_=wh[:, 0:WH_CHUNK, :, :]
        )
        for g in range(1, P // WH_CHUNK):
            sl = slice(g * WH_CHUNK, (g + 1) * WH_CHUNK)
            nc.sync.dma_start(out=wh_sb[:, sl, :, :], in_=wh[:, sl, :, :])
            if g < P // WX_CHUNK:
                slx = slice(g * WX_CHUNK, (g + 1) * WX_CHUNK)
                nc.sync.dma_start(out=wx_sb[:, slx, :, :], in_=wx[:, slx, :, :])

        h_prev = [[None, None] for _ in range(NT)]
        for p in range(P):
            ps = [
                [
                    psum.tile(
                        [128, TN],
                        dt.float32,
                        tag=f"ps_t{t}m{m}",
                        name=f"ps_p{p}t{t}m{m}",
                    )
                    for m in range(2)
                ]
                for t in range(NT)
            ]
            # K=4 wp contribution opens each accumulation group. t is
            # innermost everywhere so consecutive matmuls share lhsT (one
            # weight load covers NT matmuls). k outer / m inner for the
            # K=128 chunks: the k=0 matmuls only need the previous step's
            # m=0 relu, giving the m=1 relu a longer window to complete.
            base = 64 * (p // 16)
            for m in range(2):
                for t in range(NT):
                    nc.tensor.matmul(
                        ps[t][m][:],
                        lhsT=wx_sb[base : base + 64, p, m, :],
                        rhs=pdx_sb[base : base + 64, ts(t, TN)],
                        start=True,
                        stop=(p == 0),
                    )
            if p > 0:
                for k in range(2):
                    for m in range(2):
                        for t in range(NT):
                            nc.tensor.matmul(
                                ps[t][m][:],
                                lhsT=wh_sb[:, p, k, ts(m, 128)],
                                rhs=h_prev[t][k][:],
                                start=False,
                                stop=(k == 1),
                            )
            for t in range(NT):
                h_new = []
                for m in range(2):
                    hn = state.tile(
                        [128, TN],
                        dt.bfloat16,
                        tag=f"h_t{t}m{m}",
                        name=f"h_p{p}t{t}m{m}",
                    )
                    if m == 0:
                        nc.scalar.activation(
                            hn[:],
                            ps[t][m][:],
                            mybir.ActivationFunctionType.Relu,
                            bias=b_sb[:, p, m : m + 1],
                            scale=1.0,
                        )
                    else:
                        nc.vector.tensor_scalar(
                            hn[:],
                            ps[t][m][:],
                            scalar1=b_sb[:, p, m : m + 1],
                            scalar2=0.0,
                            op0=mybir.AluOpType.add,
                            op1=mybir.AluOpType.max,
                        )
                    nc.gpsimd.dma_start(out=out[p, m, :, ts(t, TN)], in_=hn[:])
                    h_new.append(hn)
                h_prev[t] = h_new

    nc.compile()
    return nc


def _get_nc():
    if "nc" not in _CACHE:
        _CACHE["nc"] = _build_nc()
    return _CACHE["nc"]


def _pack_inputs(path_data, W, b):
    """Host-side packing into the DRAM layouts the kernel expects."""
    # lhsT for the two K=128 chunks: wh[kk, p, k, jj] = W[p, jj, 128k+kk]
    wh_np = np.ascontiguousarray(
        W[:, :, :H].reshape(P, H, 2, 128).transpose(3, 0, 2, 1)
    ).astype(BF16)
    # zero-padded K=64 lhsT blocks for the wp chunk:
    # wx[4p+r, p, m, j] = W[p, 128m+j, 256+r]
    wx_np = np.zeros((128, P, 2, 128), dtype=BF16)
    wxs = W[:, :, H:].reshape(P, 2, 128, PD).transpose(3, 0, 1, 2).astype(BF16)
    for p in range(P):
        wx_np[4 * p : 4 * p + 4, p] = wxs[:, p]
    # bias[j, p, m] = b[p, 128m+j]
    b_np = np.ascontiguousarray(b.reshape(P, 2, 128).transpose(2, 0, 1)).astype(
        np.float32
    )
    # per-core rhs for the wp chunk: pdx[4p+r, bb] = path_data[c*BS+bb, 4p+r]
    pdx_all = [
        np.ascontiguousarray(path_data[c * BS : (c + 1) * BS].T).astype(BF16)
        for c in range(NCORES)
    ]
    return wh_np, wx_np, b_np, pdx_all


def kernel(path_data, W, b):
    from concourse.bass_utils import run_bass_kernel_spmd

    path_data = np.asarray(path_data, dtype=np.float32)
    W = np.asarray(W, dtype=np.float32)
    b = np.asarray(b, dtype=np.float32)

    wh_np, wx_np, b_np, pdx_all = _pack_inputs(path_data, W, b)
    in_maps = [
        {"wh": wh_np, "wx": wx_np, "bias": b_np, "pdx": pdx_all[c]}
        for c in range(NCORES)
    ]

    nc = _get_nc()
    res = run_bass_kernel_spmd(nc, in_maps, core_ids=list(range(NCORES)))

    full = np.concatenate(
        [
            np.asarray(r["out"])
            .transpose(3, 0, 1, 2)
            .reshape(BS, P * H)
            .astype(np.float32)
            for r in res.results
        ],
        axis=0,
    )
    return full
